# revision 15
# baseline (speedup 1.0000x reference)
"""Trainium2 Bass kernel for the dense GNN message-passing step.

Computation (N=16384, NUM_IN=1024, NUM_OUT=256):
    states = zeros(N); states[input_indices] = input_values
    total  = states @ W + biases                      # GEMV over [N, N] f32
    out    = act_select(total)[output_indices]        # 0=id, 1=relu, 2=softsign

Strategy (memory-regime roofline = bytes of W that are mathematically
needed):
  * `states` is zero outside the (<=1024) positions named by input_indices,
    so only those ROWS of W contribute to the GEMV (16x cut).
  * Only the outputs named by output_indices are returned, so only those
    COLUMNS of W are needed (64x cut). The host packs
    W[live_rows][:, output_indices] -> [1024, 256] (1 MB total), shards it
    column-wise across the 8 cores (tensor parallel, 32 outputs each =
    64 KB/core), and each core computes its GEMV slice + bias + activation
    select on-device. Core c's 32 outputs are oidx[32c:32c+32], so the
    concatenated per-core outputs ARE the gathered result.
  * fp32-exact GEMV via fp16 hi/lo decomposition: W = Wh + s*Wl,
    x = xh + s*xl with s = 2^-11; the device computes
    t = xh'Wh + s*(xl'Wh + xh'Wl) (+ bias hi/lo split the same way); the
    s^2 term (~2^-22 relative) is dropped. Products accumulate exactly in
    fp32 PSUM -> rel err ~1e-6.
  * x is stationary ([128,1] fp16 per 128-row k-chunk, 8 chunks), W is
    moving ([128,32] fp16); accumulation groups strictly sequential per
    PSUM bank: group P1 (bias_hi + xh'Wh), then group Ps (bias_lo +
    xl'Wh + xh'Wl).
  * Two input DMAs on different trigger queues: DMA1 (sync) carries
    x, bias rows, act masks and Wh -- everything the first 18 matmuls
    need; DMA2 (scalar) carries Wl, whose transfer overlaps the P1/xl*Wh
    matmuls. Descriptor generation for the two runs concurrently on the
    two sequencers.
  * Epilogue is 6 DVE-only ops on [1,32] (per-op fixed cost dominates at
    this size, and avoiding ACT skips its table load + const-AP memsets):
        t   = (Ps * s) + P1          scalar_tensor_tensor
        a1  = |t| + 1                tensor_scalar(abs_max 0, add 1)
        ss  = t / a1                 tensor_tensor(divide)
        rt  = max(t, 0)              tensor_scalar_max
        t   = m1 ? rt : t            copy_predicated (f16 0/1 mask)
        t   = m2 ? ss : t            copy_predicated
"""

import numpy as np
from contextlib import ExitStack

import concourse.bacc as bacc
import concourse.tile as tile
from concourse import mybir
from concourse.bass_utils import run_bass_kernel_spmd

N_CORES = 8
K = 1024                 # padded contraction size (live rows)
KC = K // 128            # 8 k-chunks
NOUT = 256               # gathered outputs
NPC = NOUT // N_CORES    # 32 output columns per core
S = 2.0 ** -11           # hi/lo split scale
F32 = mybir.dt.float32
F16 = mybir.dt.float16

# big1 f16 tile column layout (everything except Wl)
_XH0 = 0                 # xh columns (one per kc)
_XL0 = _XH0 + KC         # 8
_BH0 = _XL0 + KC         # 16: bias hi row (partition 0)
_BL0 = _BH0 + NPC        # 48: bias lo row
_ONE = _BL0 + NPC        # 80: constant 1.0 (bias-matmul stationary)
_M10 = _ONE + 4          # 84: relu mask (f16 0/1, partition 0)
_M20 = _M10 + NPC        # 116: softsign mask
_WH0 = _M20 + NPC + 12   # 160: Wh blocks, kc-major
C1 = _WH0 + KC * NPC     # 416 cols -> 832 B per partition
C2 = KC * NPC            # big2 = Wl [128, 256]

_BUILT = None            # cached nc so repeat calls reuse the compiled module
LAST_RESULTS = None      # BassKernelResults of the most recent run (for test.py)


def _build_bass():
    nc = bacc.Bacc(
        "TRN2", target_bir_lowering=False, debug=False, num_devices=N_CORES
    )
    b1 = nc.dram_tensor("b1", [128, C1], F16, kind="ExternalInput").ap()
    b2 = nc.dram_tensor("b2", [128, C2], F16, kind="ExternalInput").ap()
    o = nc.dram_tensor("o", [1, NPC], F32, kind="ExternalOutput").ap()

    with tile.TileContext(nc) as tc:
        with ExitStack() as ctx:
            small = ctx.enter_context(tc.tile_pool(name="small", bufs=1))
            ppool = ctx.enter_context(tc.tile_pool(name="pp", bufs=1, space="PSUM"))
            scratch = ctx.enter_context(tc.tile_pool(name="scr", bufs=1))

            b1_t = small.tile([128, C1], F16, tag="b1")
            nc.sync.dma_start(b1_t[:], b1[:])
            b2_t = small.tile([128, C2], F16, tag="b2")
            nc.scalar.dma_start(b2_t[:], b2[:])
            # masks ride b1 as f16 0/1; cast to u8 on DVE during the DMA
            # window (copy_predicated requires an integer mask dtype)
            mk_t = small.tile([1, 2 * NPC], mybir.dt.uint8, tag="mk")
            nc.vector.tensor_copy(
                mk_t[:], b1_t[0:1, _M10 : _M10 + 2 * NPC]
            )

            def wh(kc):
                return b1_t[:, _WH0 + kc * NPC : _WH0 + (kc + 1) * NPC]

            def wl(kc):
                return b2_t[:, kc * NPC : (kc + 1) * NPC]

            def xh(kc):
                return b1_t[:, _XH0 + kc : _XH0 + kc + 1]

            def xl(kc):
                return b1_t[:, _XL0 + kc : _XL0 + kc + 1]

            one = b1_t[0:1, _ONE : _ONE + 1]
            bh = b1_t[0:1, _BH0 : _BH0 + NPC]
            bl = b1_t[0:1, _BL0 : _BL0 + NPC]
            m1 = mk_t[0:1, 0:NPC]
            m2 = mk_t[0:1, NPC : 2 * NPC]

            p1 = ppool.tile([1, NPC], F32, tag="p1")
            ps = ppool.tile([1, NPC], F32, tag="ps")

            # Ps = b_lo + xl'Wh + xh'Wl  (scale S) -- emitted FIRST so the
            # DVE's Ps*S runs concurrently with the P1 matmul group
            nc.tensor.matmul(ps[0:1, :], one, bl, start=True, stop=False)
            for kc in range(KC):
                nc.tensor.matmul(
                    ps[0:1, :], xl(kc), wh(kc), start=False, stop=False
                )
            for kc in range(KC):
                nc.tensor.matmul(
                    ps[0:1, :], xh(kc), wl(kc),
                    start=False, stop=(kc == KC - 1),
                )
            # P1 = b_hi + xh'Wh  (scale 1)
            nc.tensor.matmul(p1[0:1, :], one, bh, start=True, stop=False)
            for kc in range(KC):
                nc.tensor.matmul(
                    p1[0:1, :], xh(kc), wh(kc),
                    start=False, stop=(kc == KC - 1),
                )

            # Epilogue on [1,32], interleaved across DVE and ACT so the two
            # queues overlap. (A DVE op may read only ONE input from PSUM,
            # so Ps is scaled into SBUF first; CoreV3 has no divide/abs_max
            # ALU ops, so softsign goes through Abs + reciprocal-approx.)
            ot = scratch.tile([1, NPC], F32, tag="ot")
            st = scratch.tile([1, NPC], F32, tag="st")
            a1 = scratch.tile([1, NPC], F32, tag="a1")
            at = scratch.tile([1, NPC], F32, tag="at")
            ss = scratch.tile([1, NPC], F32, tag="ss")
            rt = scratch.tile([1, NPC], F32, tag="rt")
            rf = scratch.tile([1, NPC], F32, tag="rf")
            vt = scratch.tile([1, NPC], F32, tag="vt")
            nc.vector.tensor_scalar_mul(st[:], ps[0:1, :], S)
            nc.vector.tensor_add(ot[:], p1[0:1, :], st[:])
            nc.scalar.activation(at[:], ot[:], mybir.ActivationFunctionType.Abs)
            nc.scalar.activation(                # on ACT queue before Relu so
                a1[:], at[:], mybir.ActivationFunctionType.Copy, bias=1.0
            )                                    # the reciprocal starts sooner
            nc.vector.reciprocal_approx_accurate(out=vt[:], in_=a1[:], scratch=rf[:])
            nc.scalar.activation(rt[:], ot[:], mybir.ActivationFunctionType.Relu)
            nc.vector.tensor_mul(ss[:], ot[:], vt[:])
            nc.vector.copy_predicated(ot[:], m1, rt[:])
            nc.vector.copy_predicated(ot[:], m2, ss[:])

            nc.gpsimd.dma_start(o[:], ot[:])

    nc.compile()
    return nc


def _split_f16(a):
    hi = a.astype(np.float16)
    lo = ((a - hi.astype(np.float32)) * (1.0 / S)).astype(np.float16)
    return hi, lo


def kernel(**inputs) -> np.ndarray:
    global _BUILT, LAST_RESULTS

    iv = np.asarray(inputs["input_values"], dtype=np.float32)
    W = np.asarray(inputs["weight_matrix"], dtype=np.float32)
    bias = np.asarray(inputs["biases"], dtype=np.float32)
    act = np.asarray(inputs["act_ids"])
    iidx = np.asarray(inputs["input_indices"]).astype(np.int64)
    oidx = np.asarray(inputs["output_indices"]).astype(np.int64)

    n = W.shape[0]
    # Dense neuron-state vector (duplicate indices: last write wins, matching
    # jax's .at[].set) and its index support.
    states = np.zeros(n, np.float32)
    states[iidx] = iv
    live = np.zeros(n, dtype=bool)
    live[iidx] = True
    support = np.flatnonzero(live)
    assert support.size <= K, "more than K live rows not supported"
    rows = np.zeros(K, np.int64)          # pad with row 0 (x=0 there => no-op)
    rows[: support.size] = support
    xvec = np.zeros(K, np.float32)
    xvec[: support.size] = states[support]

    Wg = W[np.ix_(rows, oidx)]            # [K, NOUT] live rows x needed cols
    bg = bias[oidx]                       # [NOUT]
    ag = act[oidx]                        # [NOUT]
    xhv, xlv = _split_f16(xvec)
    xh_t = xhv.reshape(KC, 128).T         # [128, KC]
    xl_t = xlv.reshape(KC, 128).T

    in_maps = []
    for c in range(N_CORES):
        sl = slice(c * NPC, (c + 1) * NPC)
        whc, wlc = _split_f16(Wg[:, sl])  # [K, NPC] each
        bhc, blc = _split_f16(bg[sl])
        b1a = np.zeros((128, C1), np.float16)
        b1a[:, _XH0 : _XH0 + KC] = xh_t
        b1a[:, _XL0 : _XL0 + KC] = xl_t
        b1a[0, _BH0 : _BH0 + NPC] = bhc
        b1a[0, _BL0 : _BL0 + NPC] = blc
        b1a[0, _ONE] = 1.0
        b1a[0, _M10 : _M10 + NPC] = (ag[sl] == 1).astype(np.float16)
        b1a[0, _M20 : _M20 + NPC] = (ag[sl] == 2).astype(np.float16)
        b1a[:, _WH0 : _WH0 + KC * NPC] = (
            whc.reshape(KC, 128, NPC).transpose(1, 0, 2).reshape(128, KC * NPC)
        )
        b2a = np.ascontiguousarray(
            wlc.reshape(KC, 128, NPC).transpose(1, 0, 2).reshape(128, KC * NPC)
        )
        in_maps.append({"b1": b1a, "b2": b2a})

    if _BUILT is None:
        _BUILT = _build_bass()
    LAST_RESULTS = run_bass_kernel_spmd(
        _BUILT, in_maps, core_ids=list(range(N_CORES))
    )
    full = np.concatenate(
        [LAST_RESULTS.results[c]["o"][0] for c in range(N_CORES)]
    )
    return full.astype(np.float32)


# revision 17
# speedup vs baseline: 1.2445x; 1.2445x over previous
"""Trainium2 Bass kernel for the dense GNN message-passing step.

Computation (N=16384, NUM_IN=1024, NUM_OUT=256):
    states = zeros(N); states[input_indices] = input_values
    total  = states @ W + biases                      # GEMV over [N, N] f32
    out    = act_select(total)[output_indices]        # 0=id, 1=relu, 2=softsign

Strategy (memory-regime roofline = bytes of W that are mathematically
needed):
  * `states` is zero outside the (<=1024) positions named by input_indices,
    so only those ROWS of W contribute to the GEMV (16x cut).
  * Only the outputs named by output_indices are returned, so only those
    COLUMNS of W are needed (64x cut). The host packs
    W[live_rows][:, output_indices] -> [1024, 256] (1 MB total), shards it
    column-wise across the 8 cores (tensor parallel, 32 outputs each =
    64 KB/core), and each core computes its GEMV slice + bias + activation
    select on-device. Core c's 32 outputs are oidx[32c:32c+32], so the
    concatenated per-core outputs ARE the gathered result.
  * fp32-exact GEMV via fp16 hi/lo decomposition: W = Wh + s*Wl,
    x = xh + s*xl with s = 2^-11; the device computes
    t = xh'Wh + s*(xl'Wh + xh'Wl) (+ bias hi/lo split the same way); the
    s^2 term (~2^-22 relative) is dropped. Products accumulate exactly in
    fp32 PSUM -> rel err ~1e-6.
  * x is stationary ([128,1] fp16 per 128-row k-chunk, 8 chunks), W is
    moving ([128,32] fp16); accumulation groups strictly sequential per
    PSUM bank: group P1 (bias_hi + xh'Wh), then group Ps (bias_lo +
    xl'Wh + xh'Wl).
  * Two input DMAs on different trigger queues: DMA1 (sync) carries
    x, bias rows, act masks and Wh -- everything the first 18 matmuls
    need; DMA2 (scalar) carries Wl, whose transfer overlaps the P1/xl*Wh
    matmuls. Descriptor generation for the two runs concurrently on the
    two sequencers.
  * Epilogue is 6 DVE-only ops on [1,32] (per-op fixed cost dominates at
    this size, and avoiding ACT skips its table load + const-AP memsets):
        t   = (Ps * s) + P1          scalar_tensor_tensor
        a1  = |t| + 1                tensor_scalar(abs_max 0, add 1)
        ss  = t / a1                 tensor_tensor(divide)
        rt  = max(t, 0)              tensor_scalar_max
        t   = m1 ? rt : t            copy_predicated (f16 0/1 mask)
        t   = m2 ? ss : t            copy_predicated
"""

import numpy as np
from contextlib import ExitStack

import concourse.bacc as bacc
import concourse.tile as tile
from concourse import mybir
from concourse.bass_utils import run_bass_kernel_spmd

N_CORES = 8
K = 1024                 # padded contraction size (live rows)
KC = K // 128            # 8 k-chunks
NOUT = 256               # gathered outputs
NPC = NOUT // N_CORES    # 32 output columns per core
S = 2.0 ** -11           # hi/lo split scale
F32 = mybir.dt.float32
F16 = mybir.dt.float16

# big1 f16 tile column layout (everything except Wl)
_XH0 = 0                 # xh columns (one per kc)
_XL0 = _XH0 + KC         # 8
_BH0 = _XL0 + KC         # 16: bias hi row (partition 0)
_BL0 = _BH0 + NPC        # 48: bias lo row
_ONE = _BL0 + NPC        # 80: constant 1.0 (bias-matmul stationary)
_M10 = _ONE + 4          # 84: relu mask (f16 0/1, partition 0)
_M20 = _M10 + NPC        # 116: softsign mask
_WH0 = _M20 + NPC + 12   # 160: Wh blocks, kc-major
C1 = _WH0 + KC * NPC     # 416 cols -> 832 B per partition
C2 = KC * NPC            # big2 = Wl [128, 256]

_BUILT = None            # cached nc so repeat calls reuse the compiled module
LAST_RESULTS = None      # BassKernelResults of the most recent run (for test.py)


def _build_bass():
    nc = bacc.Bacc(
        "TRN2", target_bir_lowering=False, debug=False, num_devices=N_CORES
    )
    b1 = nc.dram_tensor("b1", [128, C1], F16, kind="ExternalInput").ap()
    b2 = nc.dram_tensor("b2", [128, C2], F16, kind="ExternalInput").ap()
    o = nc.dram_tensor("o", [1, NPC], F32, kind="ExternalOutput").ap()

    with tile.TileContext(nc) as tc:
        with ExitStack() as ctx:
            small = ctx.enter_context(tc.tile_pool(name="small", bufs=1))
            ppool = ctx.enter_context(tc.tile_pool(name="pp", bufs=1, space="PSUM"))
            scratch = ctx.enter_context(tc.tile_pool(name="scr", bufs=1))

            b1_t = small.tile([128, C1], F16, tag="b1")
            nc.sync.dma_start(b1_t[:], b1[:])
            b2_t = small.tile([128, C2], F16, tag="b2")
            nc.scalar.dma_start(b2_t[:], b2[:])
            # masks ride b1 as f16 0/1; cast to u8 on DVE during the DMA
            # window (copy_predicated requires an integer mask dtype)
            mk_t = small.tile([1, 2 * NPC], mybir.dt.uint8, tag="mk")
            nc.vector.tensor_copy(
                mk_t[:], b1_t[0:1, _M10 : _M10 + 2 * NPC]
            )

            def wh(kc):
                return b1_t[:, _WH0 + kc * NPC : _WH0 + (kc + 1) * NPC]

            def wl(kc):
                return b2_t[:, kc * NPC : (kc + 1) * NPC]

            def xh(kc):
                return b1_t[:, _XH0 + kc : _XH0 + kc + 1]

            def xl(kc):
                return b1_t[:, _XL0 + kc : _XL0 + kc + 1]

            one = b1_t[0:1, _ONE : _ONE + 1]
            bh = b1_t[0:1, _BH0 : _BH0 + NPC]
            bl = b1_t[0:1, _BL0 : _BL0 + NPC]
            m1 = mk_t[0:1, 0:NPC]
            m2 = mk_t[0:1, NPC : 2 * NPC]

            p1 = ppool.tile([1, NPC], F32, tag="p1")
            ps = ppool.tile([1, NPC], F32, tag="ps")

            # P1 = b_hi + xh'Wh  (scale 1)
            nc.tensor.matmul(p1[0:1, :], one, bh, start=True, stop=False)
            for kc in range(KC):
                nc.tensor.matmul(
                    p1[0:1, :], xh(kc), wh(kc),
                    start=False, stop=(kc == KC - 1),
                )
            # Ps = b_lo + xl'Wh + xh'Wl  (scale S; the b2-dependent xh'Wl
            # matmuls come LAST so they sit behind the Wl DMA arrival)
            nc.tensor.matmul(ps[0:1, :], one, bl, start=True, stop=False)
            for kc in range(KC):
                nc.tensor.matmul(
                    ps[0:1, :], xl(kc), wh(kc), start=False, stop=False
                )
            for kc in range(KC):
                nc.tensor.matmul(
                    ps[0:1, :], xh(kc), wl(kc),
                    start=False, stop=(kc == KC - 1),
                )

            # Epilogue on [1,32], interleaved across DVE and ACT so the two
            # queues overlap. (A DVE op may read only ONE input from PSUM,
            # so Ps is scaled into SBUF first; CoreV3 has no divide/abs_max
            # ALU ops, so softsign goes through Abs + reciprocal-approx.)
            ot = scratch.tile([1, NPC], F32, tag="ot")
            st = scratch.tile([1, NPC], F32, tag="st")
            a1 = scratch.tile([1, NPC], F32, tag="a1")
            at = scratch.tile([1, NPC], F32, tag="at")
            ss = scratch.tile([1, NPC], F32, tag="ss")
            rt = scratch.tile([1, NPC], F32, tag="rt")
            rf = scratch.tile([1, NPC], F32, tag="rf")
            vt = scratch.tile([1, NPC], F32, tag="vt")
            nc.vector.tensor_scalar_mul(st[:], ps[0:1, :], S)
            nc.vector.tensor_add(ot[:], p1[0:1, :], st[:])
            nc.scalar.activation(at[:], ot[:], mybir.ActivationFunctionType.Abs)
            nc.scalar.activation(                # on ACT queue before Relu so
                a1[:], at[:], mybir.ActivationFunctionType.Copy, bias=1.0
            )                                    # the reciprocal starts sooner
            nc.vector.reciprocal_approx_accurate(out=vt[:], in_=a1[:], scratch=rf[:])
            nc.scalar.activation(rt[:], ot[:], mybir.ActivationFunctionType.Relu)
            nc.vector.tensor_mul(ss[:], ot[:], vt[:])
            nc.vector.copy_predicated(ot[:], m1, rt[:])
            nc.vector.copy_predicated(ot[:], m2, ss[:])

            nc.sync.dma_start(o[:], ot[:])

    nc.compile()
    return nc


def _split_f16(a):
    hi = a.astype(np.float16)
    lo = ((a - hi.astype(np.float32)) * (1.0 / S)).astype(np.float16)
    return hi, lo


def kernel(**inputs) -> np.ndarray:
    global _BUILT, LAST_RESULTS

    iv = np.asarray(inputs["input_values"], dtype=np.float32)
    W = np.asarray(inputs["weight_matrix"], dtype=np.float32)
    bias = np.asarray(inputs["biases"], dtype=np.float32)
    act = np.asarray(inputs["act_ids"])
    iidx = np.asarray(inputs["input_indices"]).astype(np.int64)
    oidx = np.asarray(inputs["output_indices"]).astype(np.int64)

    n = W.shape[0]
    # Dense neuron-state vector (duplicate indices: last write wins, matching
    # jax's .at[].set) and its index support.
    states = np.zeros(n, np.float32)
    states[iidx] = iv
    live = np.zeros(n, dtype=bool)
    live[iidx] = True
    support = np.flatnonzero(live)
    assert support.size <= K, "more than K live rows not supported"
    rows = np.zeros(K, np.int64)          # pad with row 0 (x=0 there => no-op)
    rows[: support.size] = support
    xvec = np.zeros(K, np.float32)
    xvec[: support.size] = states[support]

    Wg = W[np.ix_(rows, oidx)]            # [K, NOUT] live rows x needed cols
    bg = bias[oidx]                       # [NOUT]
    ag = act[oidx]                        # [NOUT]
    xhv, xlv = _split_f16(xvec)
    xh_t = xhv.reshape(KC, 128).T         # [128, KC]
    xl_t = xlv.reshape(KC, 128).T

    in_maps = []
    for c in range(N_CORES):
        sl = slice(c * NPC, (c + 1) * NPC)
        whc, wlc = _split_f16(Wg[:, sl])  # [K, NPC] each
        bhc, blc = _split_f16(bg[sl])
        b1a = np.zeros((128, C1), np.float16)
        b1a[:, _XH0 : _XH0 + KC] = xh_t
        b1a[:, _XL0 : _XL0 + KC] = xl_t
        b1a[0, _BH0 : _BH0 + NPC] = bhc
        b1a[0, _BL0 : _BL0 + NPC] = blc
        b1a[0, _ONE] = 1.0
        b1a[0, _M10 : _M10 + NPC] = (ag[sl] == 1).astype(np.float16)
        b1a[0, _M20 : _M20 + NPC] = (ag[sl] == 2).astype(np.float16)
        b1a[:, _WH0 : _WH0 + KC * NPC] = (
            whc.reshape(KC, 128, NPC).transpose(1, 0, 2).reshape(128, KC * NPC)
        )
        b2a = np.ascontiguousarray(
            wlc.reshape(KC, 128, NPC).transpose(1, 0, 2).reshape(128, KC * NPC)
        )
        in_maps.append({"b1": b1a, "b2": b2a})

    if _BUILT is None:
        _BUILT = _build_bass()
    LAST_RESULTS = run_bass_kernel_spmd(
        _BUILT, in_maps, core_ids=list(range(N_CORES))
    )
    full = np.concatenate(
        [LAST_RESULTS.results[c]["o"][0] for c in range(N_CORES)]
    )
    return full.astype(np.float32)


# revision 20
# speedup vs baseline: 1.2479x; 1.0027x over previous
"""Trainium2 Bass kernel for the dense GNN message-passing step.

Computation (N=16384, NUM_IN=1024, NUM_OUT=256):
    states = zeros(N); states[input_indices] = input_values
    total  = states @ W + biases                      # GEMV over [N, N] f32
    out    = act_select(total)[output_indices]        # 0=id, 1=relu, 2=softsign

Strategy (memory-regime roofline = bytes of W that are mathematically
needed):
  * `states` is zero outside the (<=1024) positions named by input_indices,
    so only those ROWS of W contribute to the GEMV (16x cut).
  * Only the outputs named by output_indices are returned, so only those
    COLUMNS of W are needed (64x cut). The host packs
    W[live_rows][:, output_indices] -> [1024, 256] (1 MB total), shards it
    column-wise across the 8 cores (tensor parallel, 32 outputs each =
    64 KB/core), and each core computes its GEMV slice + bias + activation
    select on-device. Core c's 32 outputs are oidx[32c:32c+32], so the
    concatenated per-core outputs ARE the gathered result.
  * fp32-exact GEMV via fp16 hi/lo decomposition: W = Wh + s*Wl,
    x = xh + s*xl with s = 2^-11; the device computes
    t = xh'Wh + s*(xl'Wh + xh'Wl) (+ bias hi/lo split the same way); the
    s^2 term (~2^-22 relative) is dropped. Products accumulate exactly in
    fp32 PSUM -> rel err ~1e-6.
  * x is stationary ([128,1] fp16 per 128-row k-chunk, 8 chunks), W is
    moving ([128,32] fp16); accumulation groups strictly sequential per
    PSUM bank: group P1 (bias_hi + xh'Wh), then group Ps (bias_lo +
    xl'Wh + xh'Wl).
  * Two input DMAs on the two HWDGE trigger queues (descriptor generation
    runs concurrently on the two sequencers): b1 (sync) carries x, bias
    rows, act masks (as f16 0/1) and Wh -- everything the first 18
    matmuls need; b2 (scalar) carries Wl, whose transfer overlaps the
    P1/xl*Wh matmuls (the b2-dependent xh'Wl matmuls are emitted last).
    The masks are cast f16->u8 on DVE during the DMA window, since
    CopyPredicated requires an integer mask and a third (u8) DMA costs
    more than the cast. The result rides one small sync-queue DMA out.
  * Epilogue on [1,32] is fixed-cost dominated (~200ns/op): kept to 9 ops
    interleaved across DVE and ACT so the queues overlap --
        st = Ps*s; t = P1+st; |t|; 1+|t| (ACT); relu (ACT, off critical
        path); r = reciprocal_approx(1+|t|); ss = t*r;
        t = m1 ? relu : t; t = m2 ? ss : t  (copy_predicated, u8 masks).
    (CoreV3 has no divide/abs_max ALU ops and CopyPredicated rejects
    float masks; a fused scalar_tensor_tensor merge of P1/Ps is illegal
    because a DVE op may read only ONE input from PSUM.)
  * Measured ~16.7-17.0 us on HW: ~7.2 us fixed NEFF/engine-boot preamble,
    ~2.4 us input DMA chain (descriptor gen + ring doorbell + transfer),
    ~1.1 us PE, ~1.9 us epilogue, ~2.2 us output DMA gen + completion,
    ~1.9 us Tile teardown (dma_reset flush + engine barriers).
"""

import numpy as np
from contextlib import ExitStack

import concourse.bacc as bacc
import concourse.tile as tile
from concourse import mybir
from concourse.bass_utils import run_bass_kernel_spmd

N_CORES = 8
K = 1024                 # padded contraction size (live rows)
KC = K // 128            # 8 k-chunks
NOUT = 256               # gathered outputs
NPC = NOUT // N_CORES    # 32 output columns per core
S = 2.0 ** -11           # hi/lo split scale
F32 = mybir.dt.float32
F16 = mybir.dt.float16

# big1 f16 tile column layout (everything except Wl)
_XH0 = 0                 # xh columns (one per kc)
_XL0 = _XH0 + KC         # 8
_BH0 = _XL0 + KC         # 16: bias hi row (partition 0)
_BL0 = _BH0 + NPC        # 48: bias lo row
_ONE = _BL0 + NPC        # 80: constant 1.0 (bias-matmul stationary)
_M10 = _ONE + 4          # 84: relu mask (f16 0/1, partition 0)
_M20 = _M10 + NPC        # 116: softsign mask
_WH0 = _M20 + NPC + 12   # 160: Wh blocks, kc-major
C1 = _WH0 + KC * NPC     # 416 cols -> 832 B per partition
C2 = KC * NPC            # big2 = Wl [128, 256]

_BUILT = None            # cached nc so repeat calls reuse the compiled module
LAST_RESULTS = None      # BassKernelResults of the most recent run (for test.py)


def _build_bass():
    nc = bacc.Bacc(
        "TRN2", target_bir_lowering=False, debug=False, num_devices=N_CORES
    )
    b1 = nc.dram_tensor("b1", [128, C1], F16, kind="ExternalInput").ap()
    b2 = nc.dram_tensor("b2", [128, C2], F16, kind="ExternalInput").ap()
    o = nc.dram_tensor("o", [1, NPC], F32, kind="ExternalOutput").ap()

    with tile.TileContext(nc) as tc:
        with ExitStack() as ctx:
            small = ctx.enter_context(tc.tile_pool(name="small", bufs=1))
            ppool = ctx.enter_context(tc.tile_pool(name="pp", bufs=1, space="PSUM"))
            scratch = ctx.enter_context(tc.tile_pool(name="scr", bufs=1))

            b1_t = small.tile([128, C1], F16, tag="b1")
            nc.sync.dma_start(b1_t[:], b1[:])
            b2_t = small.tile([128, C2], F16, tag="b2")
            nc.scalar.dma_start(b2_t[:], b2[:])
            # masks ride b1 as f16 0/1; cast to u8 on DVE during the DMA
            # window (copy_predicated requires an integer mask dtype)
            mk_t = small.tile([1, 2 * NPC], mybir.dt.uint8, tag="mk")
            nc.vector.tensor_copy(
                mk_t[:], b1_t[0:1, _M10 : _M10 + 2 * NPC]
            )

            def wh(kc):
                return b1_t[:, _WH0 + kc * NPC : _WH0 + (kc + 1) * NPC]

            def wl(kc):
                return b2_t[:, kc * NPC : (kc + 1) * NPC]

            def xh(kc):
                return b1_t[:, _XH0 + kc : _XH0 + kc + 1]

            def xl(kc):
                return b1_t[:, _XL0 + kc : _XL0 + kc + 1]

            one = b1_t[0:1, _ONE : _ONE + 1]
            bh = b1_t[0:1, _BH0 : _BH0 + NPC]
            bl = b1_t[0:1, _BL0 : _BL0 + NPC]
            m1 = mk_t[0:1, 0:NPC]
            m2 = mk_t[0:1, NPC : 2 * NPC]

            p1 = ppool.tile([1, NPC], F32, tag="p1")
            ps = ppool.tile([1, NPC], F32, tag="ps")

            # P1 = b_hi + xh'Wh  (scale 1)
            nc.tensor.matmul(p1[0:1, :], one, bh, start=True, stop=False)
            for kc in range(KC):
                nc.tensor.matmul(
                    p1[0:1, :], xh(kc), wh(kc),
                    start=False, stop=(kc == KC - 1),
                )
            # Ps = b_lo + xl'Wh + xh'Wl  (scale S; the b2-dependent xh'Wl
            # matmuls come LAST so they sit behind the Wl DMA arrival)
            nc.tensor.matmul(ps[0:1, :], one, bl, start=True, stop=False)
            for kc in range(KC):
                nc.tensor.matmul(
                    ps[0:1, :], xl(kc), wh(kc), start=False, stop=False
                )
            for kc in range(KC):
                nc.tensor.matmul(
                    ps[0:1, :], xh(kc), wl(kc),
                    start=False, stop=(kc == KC - 1),
                )

            # Epilogue on [1,32], interleaved across DVE and ACT so the two
            # queues overlap. (A DVE op may read only ONE input from PSUM,
            # so Ps is scaled into SBUF first; CoreV3 has no divide/abs_max
            # ALU ops, so softsign goes through Abs + reciprocal-approx.)
            ot = scratch.tile([1, NPC], F32, tag="ot")
            st = scratch.tile([1, NPC], F32, tag="st")
            a1 = scratch.tile([1, NPC], F32, tag="a1")
            at = scratch.tile([1, NPC], F32, tag="at")
            ss = scratch.tile([1, NPC], F32, tag="ss")
            rt = scratch.tile([1, NPC], F32, tag="rt")
            rf = scratch.tile([1, NPC], F32, tag="rf")
            vt = scratch.tile([1, NPC], F32, tag="vt")
            nc.vector.tensor_scalar_mul(st[:], ps[0:1, :], S)
            nc.vector.tensor_add(ot[:], p1[0:1, :], st[:])
            nc.scalar.activation(at[:], ot[:], mybir.ActivationFunctionType.Abs)
            nc.scalar.activation(                # on ACT queue before Relu so
                a1[:], at[:], mybir.ActivationFunctionType.Copy, bias=1.0
            )                                    # the reciprocal starts sooner
            nc.vector.reciprocal_approx_accurate(out=vt[:], in_=a1[:], scratch=rf[:])
            nc.scalar.activation(rt[:], ot[:], mybir.ActivationFunctionType.Relu)
            nc.vector.tensor_mul(ss[:], ot[:], vt[:])
            nc.vector.copy_predicated(ot[:], m1, rt[:])
            nc.vector.copy_predicated(ot[:], m2, ss[:])

            nc.sync.dma_start(o[:], ot[:])

    nc.compile()
    return nc


def _split_f16(a):
    hi = a.astype(np.float16)
    lo = ((a - hi.astype(np.float32)) * (1.0 / S)).astype(np.float16)
    return hi, lo


def kernel(**inputs) -> np.ndarray:
    global _BUILT, LAST_RESULTS

    iv = np.asarray(inputs["input_values"], dtype=np.float32)
    W = np.asarray(inputs["weight_matrix"], dtype=np.float32)
    bias = np.asarray(inputs["biases"], dtype=np.float32)
    act = np.asarray(inputs["act_ids"])
    iidx = np.asarray(inputs["input_indices"]).astype(np.int64)
    oidx = np.asarray(inputs["output_indices"]).astype(np.int64)

    n = W.shape[0]
    # Dense neuron-state vector (duplicate indices: last write wins, matching
    # jax's .at[].set) and its index support.
    states = np.zeros(n, np.float32)
    states[iidx] = iv
    live = np.zeros(n, dtype=bool)
    live[iidx] = True
    support = np.flatnonzero(live)
    assert support.size <= K, "more than K live rows not supported"
    rows = np.zeros(K, np.int64)          # pad with row 0 (x=0 there => no-op)
    rows[: support.size] = support
    xvec = np.zeros(K, np.float32)
    xvec[: support.size] = states[support]

    Wg = W[np.ix_(rows, oidx)]            # [K, NOUT] live rows x needed cols
    bg = bias[oidx]                       # [NOUT]
    ag = act[oidx]                        # [NOUT]
    xhv, xlv = _split_f16(xvec)
    xh_t = xhv.reshape(KC, 128).T         # [128, KC]
    xl_t = xlv.reshape(KC, 128).T

    in_maps = []
    for c in range(N_CORES):
        sl = slice(c * NPC, (c + 1) * NPC)
        whc, wlc = _split_f16(Wg[:, sl])  # [K, NPC] each
        bhc, blc = _split_f16(bg[sl])
        b1a = np.zeros((128, C1), np.float16)
        b1a[:, _XH0 : _XH0 + KC] = xh_t
        b1a[:, _XL0 : _XL0 + KC] = xl_t
        b1a[0, _BH0 : _BH0 + NPC] = bhc
        b1a[0, _BL0 : _BL0 + NPC] = blc
        b1a[0, _ONE] = 1.0
        b1a[0, _M10 : _M10 + NPC] = (ag[sl] == 1).astype(np.float16)
        b1a[0, _M20 : _M20 + NPC] = (ag[sl] == 2).astype(np.float16)
        b1a[:, _WH0 : _WH0 + KC * NPC] = (
            whc.reshape(KC, 128, NPC).transpose(1, 0, 2).reshape(128, KC * NPC)
        )
        b2a = np.ascontiguousarray(
            wlc.reshape(KC, 128, NPC).transpose(1, 0, 2).reshape(128, KC * NPC)
        )
        in_maps.append({"b1": b1a, "b2": b2a})

    if _BUILT is None:
        _BUILT = _build_bass()
    LAST_RESULTS = run_bass_kernel_spmd(
        _BUILT, in_maps, core_ids=list(range(N_CORES))
    )
    full = np.concatenate(
        [LAST_RESULTS.results[c]["o"][0] for c in range(N_CORES)]
    )
    return full.astype(np.float32)
